# revision 7
# baseline (speedup 1.0000x reference)
"""Causal multi-head attention (B=4, T=2048, D=1024, H=16, d_h=64) on 8 trn2 cores.

Sharding: data-parallel over batch (4) x tensor-parallel over head halves (2).
Core c handles batch c//2, heads [8*(c%2), 8*(c%2)+8), i.e. output columns
[512*(c%2), 512*(c%2)+512) of out[c//2].

v2 design — all matmul operands bf16 (rel-err budget ~2e-2 allows it; bf16
halves LDWEIGHTS time and never hits the fp32r small-N 4x penalty):

  A) x is host-cast to bf16 and DMA'd TRANSPOSED via the DMA xbar (2-byte
     dtype, [512,128] chunks) straight into xT [128, 8*2048] — no PE
     transposes, no psum->sbuf copies. V projection (v natural [t, d] with an
     interleaved ones column per head for the softmax denominator) overlaps
     the staggered xbar chunk arrivals.
  B) qT/kT [128(2 heads x d_h), 2048] per head-pair g: stationary = W tile,
     moving = xT, accumulate 8 d_in tiles in PSUM, cast-copy to bf16
     (alternating DVE/ACT).
  C) attention with 1024-wide q blocks (j2 in {0,1}), one head at a time:
     per k-tile i, diag offset m = i - 8*j2:
       scores st[:, 128m:] = kT_slice.T @ qT_slice   (one matmul, truncated
         to the causally live columns; no mask matmuls at all)
       pt = exp(st * 1/8) on ACT (bf16 out, no max subtraction: |s/8| small)
       triangular block pt[:, 128m:128m+128] *= T (0/1 lower-triangle mask)
         on DVE — fully masked prefix columns are simply never streamed
       ctxT[65, 128m:] += [v|1].T @ pt  (causally truncated stream, PSUM
         accumulate over k-tiles; emitted two k-tiles late so PE never waits
         out the exp latency; skip_group_check for the ragged accumulation)
     tail per (h, j2), chunked and paced into the next iteration's k-loop:
     ctxT -> bf16 cts (DVE) -> PE transposes into a bf16 PSUM nat tile ->
     reciprocal of the ones column -> per-partition normalize (DVE + ACT
     Copy-with-scale) -> DMA out f32.

PSUM: st ring 2x2 banks + ctxT 1x2 + nat ring 2x1 = 8 banks exactly.
"""

import os
import sys

for _p in ("/opt/trn_rl_repo", "/root/.axon_site/_ro/trn_rl_repo"):
    if os.path.isdir(_p) and _p not in sys.path:
        sys.path.insert(0, _p)

import ml_dtypes
import numpy as np

import concourse.mybir as mybir  # noqa: E402
import concourse.tile as tile  # noqa: E402
from concourse import bacc  # noqa: E402
from concourse.bass_utils import run_bass_kernel_spmd  # noqa: E402

F32 = mybir.dt.float32
BF16 = mybir.dt.bfloat16
BF = ml_dtypes.bfloat16

P = 128
T = 2048
DIN = 1024
DL = 512          # local d_out per core
HL = 8            # local heads
DH = 64
NT = T // P       # 16 t-tiles
NDI = DIN // P    # 8 d_in tiles
SCALE = 1.0 / np.sqrt(DH)

Exp = mybir.ActivationFunctionType.Exp
Copy = mybir.ActivationFunctionType.Copy


def _build():
    nc = bacc.Bacc(None, target_bir_lowering=False)
    x = nc.dram_tensor("x", [T, DIN], BF16, kind="ExternalInput")
    wq = nc.dram_tensor("wq", [DIN, DL], BF16, kind="ExternalInput")
    wk = nc.dram_tensor("wk", [DIN, DL], BF16, kind="ExternalInput")
    wv = nc.dram_tensor("wv", [DIN, DL], BF16, kind="ExternalInput")
    ident_d = nc.dram_tensor("ident", [P, P], BF16, kind="ExternalInput")
    tmask_d = nc.dram_tensor("tmask", [P, P], BF16, kind="ExternalInput")
    out = nc.dram_tensor("out", [T, DL], F32, kind="ExternalOutput")

    w_r = {n: w[:].rearrange("(k p) n -> k p n", p=P) for n, w in
           (("q", wq), ("k", wk), ("v", wv))}
    # out rows 1024*j2 + 512*u + 128*c + p
    out_r = out[:].rearrange("(j2 u c p) n -> j2 u p c n", j2=2, u=2, c=4)

    with tile.TileContext(nc) as tc:
        with (
            tc.tile_pool(name="const", bufs=1) as const,
            tc.tile_pool(name="qk", bufs=1) as qk_pool,
            tc.tile_pool(name="v", bufs=1) as v_pool,
        ):
            ident = const.tile([P, P], BF16)
            nc.sync.dma_start(out=ident, in_=ident_d[:])
            tmask = const.tile([P, P], BF16)
            nc.sync.dma_start(out=tmask, in_=tmask_d[:])
            ones_bf = const.tile([P, 2 * HL], BF16)
            nc.vector.memset(ones_bf, 1.0)
            ones2 = ones_bf[:].rearrange("p (h e) -> p h e", e=2)
            v_sb = [v_pool.tile([P, HL * (DH + 2)], BF16, tag=f"v{t_}",
                                name=f"v{t_}") for t_ in range(NT)]
            qTs = [qk_pool.tile([P, T], BF16, tag=f"qT{g}", name=f"qT{g}")
                   for g in range(4)]
            kTs = [qk_pool.tile([P, T], BF16, tag=f"kT{g}", name=f"kT{g}")
                   for g in range(4)]

            with (
                tc.tile_pool(name="xt", bufs=1) as xt_pool,
                tc.tile_pool(name="wvp", bufs=1) as wv_pool,
                tc.tile_pool(name="wqp", bufs=1) as wq_pool,
                tc.tile_pool(name="wkp", bufs=1) as wk_pool,
                tc.tile_pool(name="ps_ab", bufs=3, space="PSUM") as ps_ab,
            ):
                # ---- Phase A: xbar-transposed x DMA + V projection ----
                xT = xt_pool.tile([P, NDI * T], BF16, name="xT")
                wv_t = [wv_pool.tile([P, DL], BF16, tag=f"wv{di}",
                                     name=f"wv{di}") for di in range(NDI)]
                wq_t = [wq_pool.tile([P, DL], BF16, tag=f"wq{di}",
                                     name=f"wq{di}") for di in range(NDI)]
                wk_t = [wk_pool.tile([P, DL], BF16, tag=f"wk{di}",
                                     name=f"wk{di}") for di in range(NDI)]
                for di in range(NDI):
                    nc.sync.dma_start(out=wv_t[di], in_=w_r["v"][di])
                # x chunks t-quad-major so V proj of early t-tiles can start
                # while later chunks are still in flight
                for tq in range(4):
                    for di in range(NDI):
                        nc.sync.dma_start(
                            out=xT[:, T * di + 512 * tq:T * di + 512 * (tq + 1)],
                            in_=x[:][512 * tq:512 * (tq + 1),
                                     P * di:P * (di + 1)],
                            transpose=True)
                for di in range(NDI):
                    nc.sync.dma_start(out=wq_t[di], in_=w_r["q"][di])
                for di in range(NDI):
                    nc.sync.dma_start(out=wk_t[di], in_=w_r["k"][di])

                for t_ in range(NT):
                    psv = ps_ab.tile([P, DL], F32, tag="v", name="psv")
                    for di in range(NDI):
                        nc.tensor.matmul(
                            psv, xT[:, T * di + P * t_:T * di + P * (t_ + 1)],
                            wv_t[di], start=(di == 0), stop=(di == NDI - 1))
                    vt = v_sb[t_][:].rearrange("p (h e) -> p h e", e=DH + 2)
                    if t_ % 2 == 0:
                        nc.vector.tensor_copy(
                            vt[:, :, 0:DH],
                            psv[:].rearrange("p (h d) -> p h d", d=DH))
                        nc.scalar.copy(vt[:, :, DH:DH + 2], ones2)
                    else:
                        nc.scalar.copy(
                            vt[:, :, 0:DH],
                            psv[:].rearrange("p (h d) -> p h d", d=DH))
                        nc.vector.tensor_copy(vt[:, :, DH:DH + 2], ones2)

                # ---- Phase B2: qT/kT [128, 2048] per group g ----
                for g in range(4):
                    for which, w_t, dst in (("q", wq_t, qTs[g]),
                                            ("k", wk_t, kTs[g])):
                        for tb in range(4):
                            ps = ps_ab.tile([P, DL], F32, tag="pj",
                                            name="ps_pj")
                            for di in range(NDI):
                                nc.tensor.matmul(
                                    ps, w_t[di][:, P * g:P * (g + 1)],
                                    xT[:, T * di + 512 * tb:
                                       T * di + 512 * (tb + 1)],
                                    start=(di == 0), stop=(di == NDI - 1))
                            d_sl = dst[:, 512 * tb:512 * (tb + 1)]
                            if tb % 2 == 0:
                                nc.vector.tensor_copy(d_sl, ps)
                            else:
                                nc.scalar.copy(d_sl, ps)

            # ---- Phase C: attention ----
            with (
                tc.tile_pool(name="pt", bufs=4) as pt_pool,
                tc.tile_pool(name="cs", bufs=3) as cs_pool,
                tc.tile_pool(name="o", bufs=3) as o_pool,
                tc.tile_pool(name="ps_s", bufs=2, space="PSUM") as ps_s,
                tc.tile_pool(name="ps_ctx", bufs=1, space="PSUM") as ps_ctx,
                tc.tile_pool(name="ps_nat", bufs=2, space="PSUM") as ps_nat,
            ):
                def tail_chunks(h, j2, ctxT):
                    # Split the epilogue into small chunks paced one-per-k-tile
                    # into the next iteration. The two cts copies come first:
                    # the next iteration's first AV (start=True, same ctxT
                    # slot) must wait on them.
                    chunks = []
                    half = {}

                    def c_cts(u):
                        cts = cs_pool.tile([DH + 2, 512], BF16, tag="cts",
                                           name="cts")
                        nc.vector.tensor_copy(cts, ctxT[:, 512 * u:512 * (u + 1)])
                        half[u] = {"cts": cts}

                    def c_tr(u, lo):
                        st_ = half[u]
                        if "nat" not in st_:
                            st_["nat"] = ps_nat.tile([P, 4 * (DH + 2)], BF16,
                                                     tag="nat", name="nat")
                        nat, cts = st_["nat"], st_["cts"]
                        for c in (lo, lo + 1):
                            nc.tensor.transpose(
                                nat[:, (DH + 2) * c:(DH + 2) * (c + 1)],
                                cts[0:DH + 2, P * c:P * (c + 1)],
                                ident[0:DH + 2, 0:DH + 2])

                    def c_rec(u):
                        st_ = half[u]
                        rec = o_pool.tile([P, 4], F32, tag="rec", name="rec")
                        nc.vector.reciprocal(
                            rec, st_["nat"][:].rearrange(
                                "p (c e) -> p c e", e=DH + 2)[:, :, DH])
                        st_["rec"] = rec
                        st_["ob"] = o_pool.tile([P, 4 * DH], F32, tag="ob",
                                                name="ob")

                    def c_norm(u, lo, eng):
                        st_ = half[u]
                        nat, rec, ob = st_["nat"], st_["rec"], st_["ob"]
                        for c in (lo, lo + 1):
                            src = nat[:, (DH + 2) * c:(DH + 2) * c + DH]
                            dst = ob[:, DH * c:DH * (c + 1)]
                            if eng == 0:
                                nc.vector.tensor_scalar_mul(
                                    dst, src, st_["rec"][:, c:c + 1])
                            else:
                                nc.scalar.activation(
                                    dst, src, Copy, scale=rec[:, c:c + 1])

                    def c_out(u):
                        st_ = half[u]
                        nc.sync.dma_start(
                            out=out_r[j2][u][:, :, DH * h:DH * (h + 1)],
                            in_=st_["ob"][:].rearrange("p (c d) -> p c d",
                                                       d=DH))

                    chunks += [lambda: c_cts(0), lambda: c_cts(1)]
                    for u in range(2):
                        chunks += [
                            lambda u=u: c_tr(u, 0),
                            lambda u=u: c_tr(u, 2),
                            lambda u=u: c_rec(u),
                            lambda u=u: c_norm(u, 0, 0),
                            lambda u=u: c_norm(u, 2, 1),
                            lambda u=u: c_out(u),
                        ]
                    return chunks

                pending = []
                av_q = []
                carry = None
                order = []
                for h in range(HL):
                    order += [(h, 1), (h, 0)]
                for h, j2 in order:
                    g, sg = divmod(h, 2)
                    lo = DH * sg
                    nk = 16 if j2 == 1 else 8
                    ctxT = ps_ctx.tile([DH + 2, 1024], F32, tag="cT",
                                       name="ctxT")
                    for i in range(nk):
                        m = i - 8 * j2
                        c0 = P * m if m > 0 else 0
                        st = ps_s.tile([P, 1024], F32, tag="s", name="st")
                        # matmul outputs must stay within one PSUM bank:
                        # split into <=512-col halves
                        for v_ in range(2):
                            a = max(c0, 512 * v_)
                            b = 512 * (v_ + 1)
                            if a >= b:
                                continue
                            nc.tensor.matmul(
                                st[:, a:b],
                                kTs[g][lo:lo + DH, P * i:P * (i + 1)],
                                qTs[g][lo:lo + DH,
                                       1024 * j2 + a:1024 * j2 + b],
                                start=True, stop=True)
                        pt = pt_pool.tile([P, 1024], BF16, tag="pt", name="pt")
                        nc.scalar.activation(pt[:, c0:1024], st[:, c0:1024],
                                             Exp, scale=float(SCALE))
                        if m >= 0:
                            nc.vector.tensor_mul(pt[:, c0:c0 + P],
                                                 pt[:, c0:c0 + P], tmask)
                        if carry is not None:
                            carry()
                            carry = None
                        if pending:
                            nflush = -(-len(pending) // (nk - i))
                            for _ in range(nflush):
                                pending.pop(0)()

                        def av(i=i, pt=pt, ctxT=ctxT, h=h, c0=c0, j2=j2):
                            for v_ in range(2):
                                a = max(c0, 512 * v_)
                                b = 512 * (v_ + 1)
                                if a >= b:
                                    continue
                                nc.tensor.matmul(
                                    ctxT[:, a:b],
                                    v_sb[i][:, (DH + 2) * h:(DH + 2) * (h + 1)],
                                    pt[:, a:b],
                                    start=(i == 0),
                                    stop=(i == 8 * j2 + 4 * v_ + 3),
                                    skip_group_check=True)
                        av_q.append(av)
                        if len(av_q) > 2:
                            av_q.pop(0)()
                    # flush all but one deferred AV; the last one is emitted
                    # at the top of the next iteration's k-loop
                    while len(av_q) > 1:
                        av_q.pop(0)()
                    carry = av_q.pop(0)
                    for c in pending:
                        c()
                    pending = tail_chunks(h, j2, ctxT)
                if carry is not None:
                    carry()
                for c in pending:
                    c()
    nc.compile()
    return nc


_NC = None


def _get_nc():
    global _NC
    if _NC is None:
        _NC = _build()
    return _NC


_IDENT = np.eye(P, dtype=np.float32).astype(BF)
# T[p, u] = 1 where column u (query) >= partition p (key), else 0
_TMASK = (np.arange(P)[None, :] >= np.arange(P)[:, None]).astype(BF)


def run(inputs, **spmd_kwargs):
    x, W_q, W_k, W_v = (inputs["x"], inputs["W_q"], inputs["W_k"], inputs["W_v"])
    nc = _get_nc()
    in_maps = []
    for c in range(8):
        b, half = divmod(c, 2)
        sl = slice(DL * half, DL * half + DL)
        in_maps.append({
            "x": np.ascontiguousarray(np.asarray(x[b]).astype(BF)),
            "wq": np.ascontiguousarray(np.asarray(W_q[:, sl]).astype(BF)),
            "wk": np.ascontiguousarray(np.asarray(W_k[:, sl]).astype(BF)),
            "wv": np.ascontiguousarray(np.asarray(W_v[:, sl]).astype(BF)),
            "ident": _IDENT,
            "tmask": _TMASK,
        })
    res = run_bass_kernel_spmd(nc, in_maps, core_ids=list(range(8)), **spmd_kwargs)
    B = x.shape[0]
    full = np.empty((B, T, 2 * DL), dtype=np.float32)
    for c in range(8):
        b, half = divmod(c, 2)
        full[b][:, DL * half:DL * half + DL] = res.results[c]["out"]
    return full, res


def kernel(**inputs):
    return run(inputs)[0]


if __name__ == "__main__":
    rng = np.random.default_rng(0)
    ins = {
        "x": rng.standard_normal((4, T, DIN), dtype=np.float32),
        "W_q": (rng.random((DIN, 2 * DL), dtype=np.float32) - 0.5) / 16,
        "W_k": (rng.random((DIN, 2 * DL), dtype=np.float32) - 0.5) / 16,
        "W_v": (rng.random((DIN, 2 * DL), dtype=np.float32) - 0.5) / 16,
    }
    o = kernel(**ins)
    print("ran ok", o.shape, o.dtype)
